# revision 1
# baseline (speedup 1.0000x reference)
"""Trainium2 Bass kernel for nn_MetricNet (512-step elementwise Euler recurrence).

Strategy: pure data parallel over the batch axis — each of the 8 NeuronCores
gets 16384 frequencies laid out as a [128 partitions x 128 free] f32 tile that
lives in SBUF for the whole 512-step recurrence.

Per-step schedule (vs the 4-DVE + 3-ACT baseline at ~1740 ns/step, this runs
~1084 ns/step):

  DVE : T1  = (Ys + cb)*U              [stt]          (cb = c1+beta)
        v2' = (T1 + 2*kt)*T1           [stt == (T1+kt)^2 - kt^2]
        gg  = (v2' - (S-kt^2))*W       [stt]
        Ys' = (a2 - cb^2/2) - gg       [stt]
  ACT : a2 = Square(r2*Ys + r2*cb)     [== (Ys+cb)^2/2, bias table]
        U' = Identity(T1 + ktd)        [bias table]

The critical dependency cycle (T1 -> v2' -> gg -> Ys') runs entirely on the
DVE with no cross-engine hop: the square is an stt via (T1+2kt)*T1 with the
kt^2 correction folded into S; the quadratic Y-term arrives via the ACT's
a2 (computed from Ys at the top of the step, half a cycle ahead of its use;
its beta-corrections cancel exactly against the -cb^2/2 immediate). Each
stt is further split into two independent half-width [128,64] ops,
interleaved A/B so consecutive DVE instructions never have a direct data
hazard — this hides the write-ack wait and drops the per-op issue period
from 327 ns (dependent, full-width) to 135 ns (independent, half-width).
The Pool/GpSimd engine is unused in the loop (its SBUF traffic measurably
slows concurrent DVE instructions).

The U-state update absorbs the inv1-shift schedule (ktd = kt + delta with
delta_last = -inv1_last so the final U IS Re_out); the sigma source term
rides a host-tracked beta offset on Ys (beta' = c1*beta + beta^2/2 + sigma)
folded into per-step immediates. All per-step scalars are host-precomputed
in float64; the only SBUF tables are the two ACT bias columns (interleaved,
split-DMA'd so the first steps' columns land early). U0/Y0/W are prepared
on the host so the loop starts straight off the DMA; the final
Im = (Ys_N + beta_N)/m rescale runs on the host during unsharding.
"""

import numpy as np

import concourse.bass as bass
import concourse.mybir as mybir
import bass_rust as _br
from concourse import tile
from concourse.bass_utils import run_bass_kernel_spmd

# walrus's codegen rejects instructions carrying more than ~2 sync-wait
# commands, but Tile's exit path hangs the full end-of-kernel wait set
# (one per engine/DMA lane used) on a single SP drain. Split those waits
# across dedicated one-wait NOPs ahead of a bare drain instead.
_orig_drain_and_barrier = tile.TileContext._drain_and_barrier


def _split_drain_and_barrier(self, tick_clock, wait_clock):
    nc = self.nc
    probe = nc.sync.nop()
    wait_clock.add_sem_waits(
        probe.ins, _br.ScopedClock({None: tick_clock.global_clock})
    )
    si = probe.ins.sync_info
    if si is not None and len(si.on_wait) > 1:
        waits = list(si.on_wait)
        probe.ins.sync_info = _br.SyncInfo(
            on_wait=waits[:1], on_update=list(si.on_update)
        )
        for w in waits[1:]:
            extra = nc.sync.nop()
            extra.ins.sync_info = _br.SyncInfo(on_wait=[w], on_update=[])
    nc.sync.drain()
    nc.all_engine_barrier()
    popped = nc._tile_sem_poison_stack.pop()
    assert popped is self._sem_poison
    nc.clear_and_free_semaphores(list(self.sems.allocated().values()))
    nc.all_engine_barrier()


tile.TileContext._drain_and_barrier = _split_drain_and_barrier

# This kernel never issues PE work, but the stock all-engine barrier makes
# every engine wait ~2.6us at boot for the Tensor engine's cold wake. Run
# all full barriers over the other four engines only.
_orig_all_engine_barrier = bass.Bass.all_engine_barrier


def _no_pe_all_engine_barrier(self, *, sem_only: bool = False):
    if sem_only:
        return _orig_all_engine_barrier(self, sem_only=True)
    engs = [e for e in self.engines if e != mybir.EngineType.PE]
    self.multi_engine_barrier(engs)


bass.Bass.all_engine_barrier = _no_pe_all_engine_barrier


def _hoist_extra_waits(nc):
    """walrus's per-instruction sync-wait budget is 1 for compute/DMA
    instructions (2 for TPB_CTRL). Hoist surplus waits onto same-engine NOPs
    spliced immediately before the over-budget instruction — the engine
    executes in order, so waiting earlier is semantically identical."""
    for bb in nc.main_func.blocks:
        insts = bb.instructions
        out = []
        changed = False
        for ins in insts:
            si = ins.sync_info
            if si is not None and len(si.on_wait) > 1:
                waits = list(si.on_wait)
                for w in waits[:-1]:
                    nop = mybir.InstNoOp(
                        name=nc.get_next_instruction_name(),
                        engine=ins.engine,
                        sync_info=_br.SyncInfo(on_wait=[w], on_update=[]),
                    )
                    nc.register_instruction(nop)
                    out.append(nop)
                ins.sync_info = _br.SyncInfo(
                    on_wait=waits[-1:], on_update=list(si.on_update)
                )
                changed = True
            out.append(ins)
        if changed:
            bb.instructions = out


N_LAYERS = 512
Z_INI = 0.0
DEL_Z = 0.9 / 512.0
MU = 1.0
BATCH = 131072
N_CORES = 8
P = 128
F = BATCH // N_CORES // P  # 128

F32 = mybir.dt.float32
ALU = mybir.AluOpType
SQ = mybir.ActivationFunctionType.Square


def _host_scalars(B: np.ndarray, p: float):
    """Per-step scalar schedule, float64."""
    n = N_LAYERS
    zs = Z_INI + DEL_Z * np.arange(n, dtype=np.float64)
    b1 = B.astype(np.float64)[:n]
    b2 = B.astype(np.float64)[1 : n + 1]
    c1 = 2.0 - b2 / b1  # 1 + g
    inv1 = 1.0 / (p * (1.0 - zs))
    inv2 = inv1 / (1.0 - zs)
    kt = -DEL_Z * inv2
    delta = np.empty(n)
    delta[:-1] = inv1[1:] - inv1[:-1]
    delta[-1] = -inv1[-1]  # so the final U update yields Re_out exactly
    ktd = kt + delta
    S = -inv2 / p + inv1**2 + 1.0 / b1**2 - kt * kt
    sigma = -2.0 * DEL_Z * DEL_Z * zs * zs * (MU * MU) / b1
    beta = np.zeros(n + 1)
    for j in range(n):
        beta[j + 1] = c1[j] * beta[j] + 0.5 * beta[j] * beta[j] + sigma[j]
    return c1, kt, ktd, S, beta, inv1


def _build_bass(c1, kt, ktd, S, beta, inv1):
    n = N_LAYERS
    nc = bass.Bass()
    # packed input: [U0 | Y0 | W | a2-bias-table | ktd-table] (host-prepared)
    x_in = nc.dram_tensor("x_in", [P, 3 * F + 2 * n], F32, kind="ExternalInput")
    # packed output: [Re_out | Ys_final] (host rescales Ys -> Im)
    x_out = nc.dram_tensor("x_out", [P, 2 * F], F32, kind="ExternalOutput")

    f = float  # immediates
    with tile.TileContext(nc) as tc:
        with tc.tile_pool(name="pool", bufs=1) as pool:
            dummy = pool.tile([P, 1], F32)
            # trigger the ACT function-table load during the input DMA
            nc.scalar.activation(
                dummy[:], nc.const_aps.aps[(F32, 0.0)], SQ
            )
            xin = pool.tile([P, 3 * F + 2 * n], F32)
            tb = 3 * F
            head = min(128, 2 * n)
            nc.sync.dma_start(xin[:, 0:tb], x_in[:, 0:tb])
            # bias tables interleaved [a2_0, ktd_0, a2_1, ktd_1, ...]; split
            # so the first steps' columns land fast while the bulk streams in
            nc.scalar.dma_start(
                xin[:, tb : tb + head], x_in[:, tb : tb + head]
            )
            if 2 * n > head:
                nc.scalar.dma_start(
                    xin[:, tb + head : tb + 2 * n],
                    x_in[:, tb + head : tb + 2 * n],
                )
            U0 = xin[:, 0:F]
            Y0 = xin[:, F : 2 * F]
            W = xin[:, 2 * F : 3 * F]

            Ya = pool.tile([P, F], F32)
            Yb = pool.tile([P, F], F32)
            a2a = pool.tile([P, F], F32)
            a2b = pool.tile([P, F], F32)
            Ua = pool.tile([P, F], F32)
            Ub = pool.tile([P, F], F32)
            Ta = pool.tile([P, F], F32)
            Tb = pool.tile([P, F], F32)
            v2 = pool.tile([P, F], F32)
            gg = pool.tile([P, F], F32)
            xout = pool.tile([P, 2 * F], F32)
            reo = xout[:, 0:F]
            imo = xout[:, F : 2 * F]

            v = nc.vector
            stt = v.scalar_tensor_tensor
            Ys, Yn = Ya, Yb
            U, Un = Ua, Ub
            T1, T1n = Ta, Tb
            a2, a2n = a2a, a2b
            ID = mybir.ActivationFunctionType.Identity
            R2 = float(1.0 / np.sqrt(2.0))
            hA = slice(0, F // 2)
            hB = slice(F // 2, F)
            for j in range(n):
                cb = c1[j] + beta[j]
                ys_src = Y0 if j == 0 else Ys[:]
                u_src = U0 if j == 0 else U[:]
                yn_dst = imo if j == n - 1 else Yn[:]
                nc.scalar.activation(
                    a2[:], ys_src, SQ,
                    bias=xin[:, tb + 2 * j : tb + 2 * j + 1], scale=R2,
                )
                # each stt split into two independent half-width ops,
                # interleaved so consecutive DVE instructions never have a
                # direct data hazard (hides the write-ack wait)
                stt(T1[:, hA], ys_src[:, hA], f(cb), u_src[:, hA],
                    ALU.add, ALU.mult)
                stt(T1[:, hB], ys_src[:, hB], f(cb), u_src[:, hB],
                    ALU.add, ALU.mult)
                stt(v2[:, hA], T1[:, hA], f(2.0 * kt[j]), T1[:, hA],
                    ALU.add, ALU.mult)
                stt(v2[:, hB], T1[:, hB], f(2.0 * kt[j]), T1[:, hB],
                    ALU.add, ALU.mult)
                un_dst = reo if j == n - 1 else Un[:]
                nc.scalar.activation(
                    un_dst, T1[:], ID,
                    bias=xin[:, tb + 2 * j + 1 : tb + 2 * j + 2],
                )
                stt(gg[:, hA], v2[:, hA], f(S[j]), W[:, hA],
                    ALU.subtract, ALU.mult)
                stt(gg[:, hB], v2[:, hB], f(S[j]), W[:, hB],
                    ALU.subtract, ALU.mult)
                stt(yn_dst[:, hA], a2[:, hA], f(-0.5 * cb * cb), gg[:, hA],
                    ALU.add, ALU.subtract)
                stt(yn_dst[:, hB], a2[:, hB], f(-0.5 * cb * cb), gg[:, hB],
                    ALU.add, ALU.subtract)
                Ys, Yn = Yn, Ys
                U, Un = Un, U
                T1, T1n = T1n, T1
                a2, a2n = a2n, a2

            nc.sync.dma_start(x_out[:], xout[:])
    _hoist_extra_waits(nc)
    return nc


def kernel(Re_s, Im_s, omega, PiT, B, _trace=False):
    Re_s = np.ascontiguousarray(Re_s, dtype=np.float32)
    Im_s = np.ascontiguousarray(Im_s, dtype=np.float32)
    omega = np.ascontiguousarray(omega, dtype=np.float32)
    p = float(np.asarray(PiT).reshape(-1)[0])
    c1, kt, ktd, S, beta, inv1 = _host_scalars(np.asarray(B), p)

    nc = _build_bass(c1, kt, ktd, S, beta, inv1)

    m64 = 2.0 * DEL_Z * omega.astype(np.float64)
    U0 = (Re_s.astype(np.float64) + inv1[0]).astype(np.float32)
    Y0 = (Im_s.astype(np.float64) * m64).astype(np.float32)
    Wf = (0.5 * m64 * m64).astype(np.float32)
    U08 = U0.reshape(N_CORES, P, F)
    Y08 = Y0.reshape(N_CORES, P, F)
    W8 = Wf.reshape(N_CORES, P, F)
    cb = c1 + beta[:N_LAYERS]
    tabs = np.empty(2 * N_LAYERS)
    tabs[0::2] = cb / np.sqrt(2.0)
    tabs[1::2] = ktd
    tabs = tabs.astype(np.float32)  # interleaved ACT bias tables
    tab8 = np.broadcast_to(tabs, (N_CORES, P, tabs.size))
    xin = np.concatenate([U08, Y08, W8, tab8], axis=2)  # [8, P, 3F+2n]
    in_maps = [{"x_in": np.ascontiguousarray(xin[i])} for i in range(N_CORES)]
    res = run_bass_kernel_spmd(nc, in_maps, list(range(N_CORES)), trace=_trace)
    re_full = np.concatenate(
        [res.results[i]["x_out"][:, 0:F].reshape(-1) for i in range(N_CORES)]
    )
    ys_full = np.concatenate(
        [res.results[i]["x_out"][:, F : 2 * F].reshape(-1) for i in range(N_CORES)]
    )
    im_full = (ys_full.astype(np.float64) + beta[N_LAYERS]) / m64
    if _trace:
        kernel.last_results = res
    return re_full.astype(np.float32), im_full.astype(np.float32)



# revision 2
# speedup vs baseline: 1.3164x; 1.3164x over previous
"""Trainium2 Bass kernel for nn_MetricNet (512-step elementwise Euler recurrence).

Strategy: pure data parallel over the batch axis — each of the 8 NeuronCores
gets 16384 frequencies as a [128 x 128] f32 SBUF tile held for all 512 steps.

The recurrence is reduced to THREE DVE ops per step using two custom DVE
uop programs (the stock formulation needs four: T1, v2, gg, Ys):

  T1_j      = (Yc_j + d_j) * (T1_{j-1} + ktd_{j-1})   [custom TT_AA]
  GG_j      = (sq(T1_j + kt_j) - S0_j) * W            [custom QGW]
  Yc_{j+1}  = a2_j - GG_j                             [tensor_tensor sub]
  a2_j      = Sq(r2*Yc_j + r2*d_j)                    [ACT, off-chain]

TT_AA folds the U-state update away entirely (U_{j+1} = T1_j + ktd_j is
substituted into the next step's product, so U never materialises); QGW
fuses the square, the S0 shift and the W multiply (S0 = 1/b1^2 exactly —
the inv1^2 and inv2/p terms cancel). The Yc gauge (Y_j = Yc_j + e_j with
e_{j+1} = sigma_j - c1_j^2/2, d_j = e_j + c1_j) absorbs every additive
per-step constant so the Yc update is a bare subtract reading the ACT's
a2. Each op is split into two independent half-width [128,64] ops,
interleaved A/B so consecutive DVE instructions never have a direct data
hazard; measured issue is gap-free at ~136 ns/slice -> ~820 ns/step vs
1085 ns/step for the 4-op baseline.

Final outputs are assembled on the host in f64 during unsharding:
Re = T1_last + ktd_last (the last ktd carries delta = -inv1 so this IS
Re), Im = (Yc_final + e_final)/m with m = 2*dz*omega.
"""

import numpy as np

import concourse.bass as bass
import concourse.mybir as mybir
import bass_rust as _br
from concourse import tile
from concourse.bass_utils import run_bass_kernel_spmd

# walrus's codegen rejects instructions carrying more than ~2 sync-wait
# commands, but Tile's exit path hangs the full end-of-kernel wait set
# (one per engine/DMA lane used) on a single SP drain. Split those waits
# across dedicated one-wait NOPs ahead of a bare drain instead.
_orig_drain_and_barrier = tile.TileContext._drain_and_barrier


def _split_drain_and_barrier(self, tick_clock, wait_clock):
    nc = self.nc
    probe = nc.sync.nop()
    wait_clock.add_sem_waits(
        probe.ins, _br.ScopedClock({None: tick_clock.global_clock})
    )
    si = probe.ins.sync_info
    if si is not None and len(si.on_wait) > 1:
        waits = list(si.on_wait)
        probe.ins.sync_info = _br.SyncInfo(
            on_wait=waits[:1], on_update=list(si.on_update)
        )
        for w in waits[1:]:
            extra = nc.sync.nop()
            extra.ins.sync_info = _br.SyncInfo(on_wait=[w], on_update=[])
    nc.sync.drain()
    nc.all_engine_barrier()
    popped = nc._tile_sem_poison_stack.pop()
    assert popped is self._sem_poison
    nc.clear_and_free_semaphores(list(self.sems.allocated().values()))
    nc.all_engine_barrier()


tile.TileContext._drain_and_barrier = _split_drain_and_barrier

# This kernel never issues PE work, but the stock all-engine barrier makes
# every engine wait ~2.6us at boot for the Tensor engine's cold wake. Run
# all full barriers over the other four engines only.
_orig_all_engine_barrier = bass.Bass.all_engine_barrier


def _no_pe_all_engine_barrier(self, *, sem_only: bool = False):
    if sem_only:
        return _orig_all_engine_barrier(self, sem_only=True)
    engs = [e for e in self.engines if e != mybir.EngineType.PE]
    self.multi_engine_barrier(engs)


bass.Bass.all_engine_barrier = _no_pe_all_engine_barrier


def _hoist_extra_waits(nc):
    """walrus's per-instruction sync-wait budget is 1 for compute/DMA
    instructions (2 for TPB_CTRL). Hoist surplus waits onto same-engine NOPs
    spliced immediately before the over-budget instruction — the engine
    executes in order, so waiting earlier is semantically identical."""
    for bb in nc.main_func.blocks:
        insts = bb.instructions
        out = []
        changed = False
        for ins in insts:
            si = ins.sync_info
            if si is not None and len(si.on_wait) > 1:
                waits = list(si.on_wait)
                for w in waits[:-1]:
                    nop = mybir.InstNoOp(
                        name=nc.get_next_instruction_name(),
                        engine=ins.engine,
                        sync_info=_br.SyncInfo(on_wait=[w], on_update=[]),
                    )
                    nc.register_instruction(nop)
                    out.append(nop)
                ins.sync_info = _br.SyncInfo(
                    on_wait=waits[-1:], on_update=list(si.on_update)
                )
                changed = True
            out.append(ins)
        if changed:
            bb.instructions = out


def _register_custom_ops():
    """Register the two fused DVE uop programs (idempotent)."""
    import concourse.dve_ops as dve_ops
    from concourse.dve_spec import Spec, Src0, Src1, C0, C1, lower, sq
    from concourse.dve_spec import _has_src1 as has_src1
    from concourse.dve_uop import DveOpSpec

    existing = {op.name: op for op in dve_ops.OPS}
    if "ANT_TT_AA" in existing:
        return existing["ANT_TT_AA"], existing["ANT_QGW"]

    def make(name, body, ref):
        spec = Spec(body=body, reference=ref)
        row = dve_ops._CUSTOM_DVE_ROW_BASE + len(dve_ops.OPS)
        shas = {}
        for ver in ("v3", "v4"):
            try:
                uops = lower(spec, ver=ver)
                shas[ver] = DveOpSpec(
                    name=name, opcode=row, uops=uops, rd1_en=has_src1(spec)
                ).sha(ver)
            except Exception:
                pass
        op = dve_ops.DveOp(name, spec, subdim=False, uops_sha=shas)
        dve_ops.OPS.append(op)
        dve_ops._SUB_OPCODE_FOR_NAME[name] = row
        dve_ops.CUSTOM_DVE_SPECS[name] = spec
        return op

    tt_aa = make(
        "ANT_TT_AA",
        (Src0 + C0) * (Src1 + C1),
        lambda in0, in1, s0, s1, imm2: (in0.astype(np.float32) + s0)
        * (in1 + s1),
    )
    qgw = make(
        "ANT_QGW",
        (sq(Src0 + C0) + C1) * Src1,
        lambda in0, in1, s0, s1, imm2: (
            ((in0.astype(np.float32) + s0) ** 2 + s1) * in1
        ),
    )
    return tt_aa, qgw


N_LAYERS = 512
Z_INI = 0.0
DEL_Z = 0.9 / 512.0
MU = 1.0
BATCH = 131072
N_CORES = 8
P = 128
F = BATCH // N_CORES // P  # 128

F32 = mybir.dt.float32
ALU = mybir.AluOpType
SQ = mybir.ActivationFunctionType.Square
R2 = float(1.0 / np.sqrt(2.0))


def _host_scalars(B: np.ndarray, p: float):
    """Per-step scalar schedule, float64."""
    n = N_LAYERS
    zs = Z_INI + DEL_Z * np.arange(n, dtype=np.float64)
    b1 = B.astype(np.float64)[:n]
    b2 = B.astype(np.float64)[1 : n + 1]
    c1 = 2.0 - b2 / b1  # 1 + g
    inv1 = 1.0 / (p * (1.0 - zs))
    inv2 = inv1 / (1.0 - zs)
    kt = -DEL_Z * inv2
    delta = np.empty(n)
    delta[:-1] = inv1[1:] - inv1[:-1]
    delta[-1] = -inv1[-1]  # so the final U update yields Re_out exactly
    ktd = kt + delta
    # S0 = -inv2/p + inv1^2 + 1/b1^2; the first two cancel exactly
    S0 = 1.0 / (b1 * b1)
    sigma = -2.0 * DEL_Z * DEL_Z * zs * zs * (MU * MU) / b1
    e = np.zeros(n + 1)
    e[1:] = sigma - 0.5 * c1 * c1  # gauge offset: Y_j = Yc_j + e_j
    d = e[:n] + c1
    return c1, kt, ktd, S0, e, d, inv1


def _build_bass(tt_aa, qgw, d, kt, ktd, S0):
    n = N_LAYERS
    nc = bass.Bass()
    # packed input: [U0 | Yc0 | W | a2-bias-table] (host-prepared)
    x_in = nc.dram_tensor("x_in", [P, 3 * F + n], F32, kind="ExternalInput")
    # packed output: [T1_last | Yc_final] (host assembles Re/Im)
    x_out = nc.dram_tensor("x_out", [P, 2 * F], F32, kind="ExternalOutput")

    f = float  # immediates
    with tile.TileContext(nc) as tc:
        with tc.tile_pool(name="pool", bufs=1) as pool:
            dummy = pool.tile([P, 1], F32)
            # trigger the ACT function-table load during the input DMA
            nc.scalar.activation(
                dummy[:], nc.const_aps.aps[(F32, 0.0)], SQ
            )
            xin = pool.tile([P, 3 * F + n], F32)
            tb = 3 * F
            head = min(128, n)
            nc.sync.dma_start(xin[:, 0:tb], x_in[:, 0:tb])
            # bias table split so the first steps' columns land fast
            nc.scalar.dma_start(xin[:, tb : tb + head], x_in[:, tb : tb + head])
            if n > head:
                nc.scalar.dma_start(
                    xin[:, tb + head : tb + n], x_in[:, tb + head : tb + n]
                )
            U0 = xin[:, 0:F]
            Y0 = xin[:, F : 2 * F]
            W = xin[:, 2 * F : 3 * F]

            Ya = pool.tile([P, F], F32)
            Yb = pool.tile([P, F], F32)
            Ta = pool.tile([P, F], F32)
            Tb = pool.tile([P, F], F32)
            GG = pool.tile([P, F], F32)
            a2a = pool.tile([P, F], F32)
            a2b = pool.tile([P, F], F32)
            xout = pool.tile([P, 2 * F], F32)
            t1o = xout[:, 0:F]
            yco = xout[:, F : 2 * F]

            v = nc.vector
            hA = slice(0, F // 2)
            hB = slice(F // 2, F)
            Ys, Yn = Ya, Yb
            T1, T1p = Ta, Tb
            a2, a2n = a2a, a2b
            for j in range(n):
                ys_src = Y0 if j == 0 else Ys[:]
                t1p_src = U0 if j == 0 else T1p[:]
                ktd_prev = 0.0 if j == 0 else f(ktd[j - 1])
                t1_dst = t1o if j == n - 1 else T1[:]
                yn_dst = yco if j == n - 1 else Yn[:]
                bias = xin[:, tb + j : tb + j + 1]
                # ACT: a2 = Sq(r2*Yc + r2*d_j), split A/B halves
                nc.scalar.activation(a2[:, hA], ys_src[:, hA], SQ,
                                     bias=bias, scale=R2)
                nc.scalar.activation(a2[:, hB], ys_src[:, hB], SQ,
                                     bias=bias, scale=R2)
                # DVE: T1 = (Yc + d_j)*(T1p + ktd_{j-1})
                v._custom_dve(tt_aa, out=t1_dst[:, hA], in0=ys_src[:, hA],
                              in1=t1p_src[:, hA], s0=f(d[j]), s1=ktd_prev)
                v._custom_dve(tt_aa, out=t1_dst[:, hB], in0=ys_src[:, hB],
                              in1=t1p_src[:, hB], s0=f(d[j]), s1=ktd_prev)
                # DVE: GG = (sq(T1 + kt_j) - S0_j)*W
                v._custom_dve(qgw, out=GG[:, hA], in0=t1_dst[:, hA],
                              in1=W[:, hA], s0=f(kt[j]), s1=f(-S0[j]))
                v._custom_dve(qgw, out=GG[:, hB], in0=t1_dst[:, hB],
                              in1=W[:, hB], s0=f(kt[j]), s1=f(-S0[j]))
                # DVE: Yc' = a2 - GG
                v.tensor_tensor(yn_dst[:, hA], a2[:, hA], GG[:, hA], ALU.subtract)
                v.tensor_tensor(yn_dst[:, hB], a2[:, hB], GG[:, hB], ALU.subtract)
                Ys, Yn = Yn, Ys
                T1, T1p = T1p, T1
                a2, a2n = a2n, a2

            nc.sync.dma_start(x_out[:], xout[:])
    _hoist_extra_waits(nc)
    mybir.codegen_inst_isa_subclasses(nc)
    return nc


def kernel(Re_s, Im_s, omega, PiT, B, _trace=False):
    Re_s = np.ascontiguousarray(Re_s, dtype=np.float32)
    Im_s = np.ascontiguousarray(Im_s, dtype=np.float32)
    omega = np.ascontiguousarray(omega, dtype=np.float32)
    p = float(np.asarray(PiT).reshape(-1)[0])
    n = N_LAYERS
    tt_aa, qgw = _register_custom_ops()
    c1, kt, ktd, S0, e, d, inv1 = _host_scalars(np.asarray(B), p)

    nc = _build_bass(tt_aa, qgw, d, kt, ktd, S0)

    m64 = 2.0 * DEL_Z * omega.astype(np.float64)
    U0 = (Re_s.astype(np.float64) + inv1[0]).astype(np.float32)
    Y0 = (Im_s.astype(np.float64) * m64).astype(np.float32)
    Wf = (0.5 * m64 * m64).astype(np.float32)
    U08 = U0.reshape(N_CORES, P, F)
    Y08 = Y0.reshape(N_CORES, P, F)
    W8 = Wf.reshape(N_CORES, P, F)
    tabs = (R2 * d[:n]).astype(np.float32)  # a2 bias table
    tab8 = np.broadcast_to(tabs, (N_CORES, P, n))
    xin = np.concatenate([U08, Y08, W8, tab8], axis=2)  # [8, P, 3F+n]
    in_maps = [{"x_in": np.ascontiguousarray(xin[i])} for i in range(N_CORES)]
    res = run_bass_kernel_spmd(nc, in_maps, list(range(N_CORES)), trace=_trace)
    t1_full = np.concatenate(
        [res.results[i]["x_out"][:, 0:F].reshape(-1) for i in range(N_CORES)]
    )
    yc_full = np.concatenate(
        [res.results[i]["x_out"][:, F : 2 * F].reshape(-1) for i in range(N_CORES)]
    )
    re_full = t1_full.astype(np.float64) + ktd[n - 1]
    im_full = (yc_full.astype(np.float64) + e[n]) / m64
    if _trace:
        kernel.last_results = res
    return re_full.astype(np.float32), im_full.astype(np.float32)


# revision 3
# speedup vs baseline: 1.3274x; 1.0084x over previous
"""Trainium2 Bass kernel for nn_MetricNet (512-step elementwise Euler recurrence).

Strategy: pure data parallel over the batch axis — each of the 8 NeuronCores
gets 16384 frequencies as a [128 x 128] f32 SBUF tile held for all 512 steps.

The recurrence runs as THREE custom DVE uop programs per step (the stock
ALU set needs four DVE ops plus two ACT ops):

  T1_j     = (Yc_j + d_j) * (T1_{j-1} + ktd_{j-1})    [ANT_TT_AA]
  GG_j     = (sq(T1_j + kt_j) - S0_j) * W             [ANT_QGW]
  Yc_{j+1} = sq(Yc_j + d_j) * 0.5 - GG_j              [ANT_YSQ]

ANT_TT_AA folds the U-state update away entirely (U_{j+1} = T1_j + ktd_j
is substituted into the next step's product, so U never materialises);
ANT_QGW fuses the square, the S0 shift and the W multiply (S0 = 1/b1^2
exactly — the inv1^2 and inv2/p terms cancel); ANT_YSQ fuses the
(Y+c1)^2/2 term so the Scalar engine drops out of the loop completely.
The Yc gauge (Y_j = Yc_j + e_j with e_{j+1} = sigma_j - c1_j^2/2,
d_j = e_j + c1_j) absorbs every additive per-step constant. Each op is
split into two independent half-width [128,64] ops, interleaved A/B so
consecutive DVE instructions never have a direct data hazard. The loop
is single-engine: no semaphores, 3072 back-to-back DVE instructions.

Final outputs are assembled on the host in f64 during unsharding:
Re = T1_last + ktd_last (the last ktd carries delta = -inv1 so this IS
Re), Im = (Yc_final + e_final)/m with m = 2*dz*omega.
"""

import numpy as np

import concourse.bass as bass
import concourse.mybir as mybir
import bass_rust as _br
from concourse import tile
from concourse.bass_utils import run_bass_kernel_spmd

# walrus's codegen rejects instructions carrying more than ~2 sync-wait
# commands, but Tile's exit path hangs the full end-of-kernel wait set
# (one per engine/DMA lane used) on a single SP drain. Split those waits
# across dedicated one-wait NOPs ahead of a bare drain instead.
_orig_drain_and_barrier = tile.TileContext._drain_and_barrier


def _split_drain_and_barrier(self, tick_clock, wait_clock):
    nc = self.nc
    probe = nc.sync.nop()
    wait_clock.add_sem_waits(
        probe.ins, _br.ScopedClock({None: tick_clock.global_clock})
    )
    si = probe.ins.sync_info
    if si is not None and len(si.on_wait) > 1:
        waits = list(si.on_wait)
        probe.ins.sync_info = _br.SyncInfo(
            on_wait=waits[:1], on_update=list(si.on_update)
        )
        for w in waits[1:]:
            extra = nc.sync.nop()
            extra.ins.sync_info = _br.SyncInfo(on_wait=[w], on_update=[])
    nc.sync.drain()
    nc.all_engine_barrier()
    popped = nc._tile_sem_poison_stack.pop()
    assert popped is self._sem_poison
    nc.clear_and_free_semaphores(list(self.sems.allocated().values()))
    nc.all_engine_barrier()


tile.TileContext._drain_and_barrier = _split_drain_and_barrier

# This kernel only uses DVE (+SP for DMA), but the stock all-engine barrier
# makes every engine wait ~2.6us at boot for the Tensor engine's cold wake.
# Run all full barriers over the non-PE engines only.
_orig_all_engine_barrier = bass.Bass.all_engine_barrier


def _no_pe_all_engine_barrier(self, *, sem_only: bool = False):
    if sem_only:
        return _orig_all_engine_barrier(self, sem_only=True)
    engs = [e for e in self.engines if e != mybir.EngineType.PE]
    self.multi_engine_barrier(engs)


bass.Bass.all_engine_barrier = _no_pe_all_engine_barrier


def _hoist_extra_waits(nc):
    """walrus's per-instruction sync-wait budget is 1 for compute/DMA
    instructions (2 for TPB_CTRL). Hoist surplus waits onto same-engine NOPs
    spliced immediately before the over-budget instruction — the engine
    executes in order, so waiting earlier is semantically identical."""
    for bb in nc.main_func.blocks:
        insts = bb.instructions
        out = []
        changed = False
        for ins in insts:
            si = ins.sync_info
            if si is not None and len(si.on_wait) > 1:
                waits = list(si.on_wait)
                for w in waits[:-1]:
                    nop = mybir.InstNoOp(
                        name=nc.get_next_instruction_name(),
                        engine=ins.engine,
                        sync_info=_br.SyncInfo(on_wait=[w], on_update=[]),
                    )
                    nc.register_instruction(nop)
                    out.append(nop)
                ins.sync_info = _br.SyncInfo(
                    on_wait=waits[-1:], on_update=list(si.on_update)
                )
                changed = True
            out.append(ins)
        if changed:
            bb.instructions = out


def _register_custom_ops():
    """Register the three fused DVE uop programs (idempotent)."""
    import concourse.dve_ops as dve_ops
    from concourse.dve_spec import Spec, Src0, Src1, C0, C1, lower, sq
    from concourse.dve_spec import _has_src1 as has_src1
    from concourse.dve_uop import DveOpSpec

    existing = {op.name: op for op in dve_ops.OPS}
    if "ANT_TT_AA" in existing:
        return (existing["ANT_TT_AA"], existing["ANT_QGW"],
                existing["ANT_YSQ"])

    def make(name, body, ref):
        spec = Spec(body=body, reference=ref)
        row = dve_ops._CUSTOM_DVE_ROW_BASE + len(dve_ops.OPS)
        shas = {}
        for ver in ("v3", "v4"):
            try:
                uops = lower(spec, ver=ver)
                shas[ver] = DveOpSpec(
                    name=name, opcode=row, uops=uops, rd1_en=has_src1(spec)
                ).sha(ver)
            except Exception:
                pass
        op = dve_ops.DveOp(name, spec, subdim=False, uops_sha=shas)
        dve_ops.OPS.append(op)
        dve_ops._SUB_OPCODE_FOR_NAME[name] = row
        dve_ops.CUSTOM_DVE_SPECS[name] = spec
        return op

    tt_aa = make(
        "ANT_TT_AA",
        (Src0 + C0) * (Src1 + C1),
        lambda in0, in1, s0, s1, imm2: (in0.astype(np.float32) + s0)
        * (in1 + s1),
    )
    qgw = make(
        "ANT_QGW",
        (sq(Src0 + C0) + C1) * Src1,
        lambda in0, in1, s0, s1, imm2: (
            ((in0.astype(np.float32) + s0) ** 2 + s1) * in1
        ),
    )
    ysq = make(
        "ANT_YSQ",
        sq(Src0 + C0) * C1 - Src1,
        lambda in0, in1, s0, s1, imm2: (
            (in0.astype(np.float32) + s0) ** 2 * s1 - in1
        ),
    )
    return tt_aa, qgw, ysq


N_LAYERS = 512
Z_INI = 0.0
DEL_Z = 0.9 / 512.0
MU = 1.0
BATCH = 131072
N_CORES = 8
P = 128
F = BATCH // N_CORES // P  # 128

F32 = mybir.dt.float32
ALU = mybir.AluOpType


def _host_scalars(B: np.ndarray, p: float):
    """Per-step scalar schedule, float64."""
    n = N_LAYERS
    zs = Z_INI + DEL_Z * np.arange(n, dtype=np.float64)
    b1 = B.astype(np.float64)[:n]
    b2 = B.astype(np.float64)[1 : n + 1]
    c1 = 2.0 - b2 / b1  # 1 + g
    inv1 = 1.0 / (p * (1.0 - zs))
    inv2 = inv1 / (1.0 - zs)
    kt = -DEL_Z * inv2
    delta = np.empty(n)
    delta[:-1] = inv1[1:] - inv1[:-1]
    delta[-1] = -inv1[-1]  # so the final U update yields Re_out exactly
    ktd = kt + delta
    # S0 = -inv2/p + inv1^2 + 1/b1^2; the first two cancel exactly
    S0 = 1.0 / (b1 * b1)
    sigma = -2.0 * DEL_Z * DEL_Z * zs * zs * (MU * MU) / b1
    e = np.zeros(n + 1)
    e[1:] = sigma - 0.5 * c1 * c1  # gauge offset: Y_j = Yc_j + e_j
    d = e[:n] + c1
    return c1, kt, ktd, S0, e, d, inv1


def _build_bass(tt_aa, qgw, ysq, d, kt, ktd, S0):
    n = N_LAYERS
    nc = bass.Bass()
    # packed input: [U0 | Yc0 | W] (host-prepared)
    x_in = nc.dram_tensor("x_in", [P, 3 * F], F32, kind="ExternalInput")
    # packed output: [T1_last | Yc_final] (host assembles Re/Im)
    x_out = nc.dram_tensor("x_out", [P, 2 * F], F32, kind="ExternalOutput")

    f = float  # immediates
    with tile.TileContext(nc) as tc:
        with tc.tile_pool(name="pool", bufs=1) as pool:
            xin = pool.tile([P, 3 * F], F32)
            nc.sync.dma_start(xin[:], x_in[:])
            U0 = xin[:, 0:F]
            Y0 = xin[:, F : 2 * F]
            W = xin[:, 2 * F : 3 * F]

            Ya = pool.tile([P, F], F32)
            Yb = pool.tile([P, F], F32)
            Ta = pool.tile([P, F], F32)
            Tb = pool.tile([P, F], F32)
            GG = pool.tile([P, F], F32)
            xout = pool.tile([P, 2 * F], F32)
            t1o = xout[:, 0:F]
            yco = xout[:, F : 2 * F]

            v = nc.vector
            hA = slice(0, F // 2)
            hB = slice(F // 2, F)
            Ys, Yn = Ya, Yb
            T1, T1p = Ta, Tb
            for j in range(n):
                ys_src = Y0 if j == 0 else Ys[:]
                t1p_src = U0 if j == 0 else T1p[:]
                ktd_prev = 0.0 if j == 0 else f(ktd[j - 1])
                t1_dst = t1o if j == n - 1 else T1[:]
                yn_dst = yco if j == n - 1 else Yn[:]
                dj = f(d[j])
                # T1 = (Yc + d_j)*(T1p + ktd_{j-1})
                v._custom_dve(tt_aa, out=t1_dst[:, hA], in0=ys_src[:, hA],
                              in1=t1p_src[:, hA], s0=dj, s1=ktd_prev)
                v._custom_dve(tt_aa, out=t1_dst[:, hB], in0=ys_src[:, hB],
                              in1=t1p_src[:, hB], s0=dj, s1=ktd_prev)
                # GG = (sq(T1 + kt_j) - S0_j)*W
                v._custom_dve(qgw, out=GG[:, hA], in0=t1_dst[:, hA],
                              in1=W[:, hA], s0=f(kt[j]), s1=f(-S0[j]))
                v._custom_dve(qgw, out=GG[:, hB], in0=t1_dst[:, hB],
                              in1=W[:, hB], s0=f(kt[j]), s1=f(-S0[j]))
                # Yc' = sq(Yc + d_j)*0.5 - GG
                v._custom_dve(ysq, out=yn_dst[:, hA], in0=ys_src[:, hA],
                              in1=GG[:, hA], s0=dj, s1=0.5)
                v._custom_dve(ysq, out=yn_dst[:, hB], in0=ys_src[:, hB],
                              in1=GG[:, hB], s0=dj, s1=0.5)
                Ys, Yn = Yn, Ys
                T1, T1p = T1p, T1

            nc.sync.dma_start(x_out[:], xout[:])
    _hoist_extra_waits(nc)
    mybir.codegen_inst_isa_subclasses(nc)
    return nc


def kernel(Re_s, Im_s, omega, PiT, B, _trace=False):
    Re_s = np.ascontiguousarray(Re_s, dtype=np.float32)
    Im_s = np.ascontiguousarray(Im_s, dtype=np.float32)
    omega = np.ascontiguousarray(omega, dtype=np.float32)
    p = float(np.asarray(PiT).reshape(-1)[0])
    n = N_LAYERS
    tt_aa, qgw, ysq = _register_custom_ops()
    c1, kt, ktd, S0, e, d, inv1 = _host_scalars(np.asarray(B), p)

    nc = _build_bass(tt_aa, qgw, ysq, d, kt, ktd, S0)

    m64 = 2.0 * DEL_Z * omega.astype(np.float64)
    U0 = (Re_s.astype(np.float64) + inv1[0]).astype(np.float32)
    Y0 = (Im_s.astype(np.float64) * m64).astype(np.float32)
    Wf = (0.5 * m64 * m64).astype(np.float32)
    U08 = U0.reshape(N_CORES, P, F)
    Y08 = Y0.reshape(N_CORES, P, F)
    W8 = Wf.reshape(N_CORES, P, F)
    xin = np.concatenate([U08, Y08, W8], axis=2)  # [8, P, 3F]
    in_maps = [{"x_in": np.ascontiguousarray(xin[i])} for i in range(N_CORES)]
    res = run_bass_kernel_spmd(nc, in_maps, list(range(N_CORES)), trace=_trace)
    t1_full = np.concatenate(
        [res.results[i]["x_out"][:, 0:F].reshape(-1) for i in range(N_CORES)]
    )
    yc_full = np.concatenate(
        [res.results[i]["x_out"][:, F : 2 * F].reshape(-1) for i in range(N_CORES)]
    )
    re_full = t1_full.astype(np.float64) + ktd[n - 1]
    im_full = (yc_full.astype(np.float64) + e[n]) / m64
    if _trace:
        kernel.last_results = res
    return re_full.astype(np.float32), im_full.astype(np.float32)
